# revision 35
# baseline (speedup 1.0000x reference)
"""Cross-attention block (nn_CABlock) on 8 TRN2 NeuronCores.

Reference (per batch b):
    q  = xq @ Wq.T            -> [SQ, H]   split heads [SQ, 16, 64]
    kv = xkv @ Wkv.T          -> [SKV, 2H] split [SKV, 2, 16, 64]
    att = softmax(q k^T / sqrt(64))
    x   = att @ v  (merge heads)
    out = x @ Wout.T + bout

Sharding: 8 cores = 4 batches x 2 head-groups (8 heads each).  Each core
computes its batch's projections restricted to its 8 heads, attention for
those heads, and a partial out-projection (contraction over its 512
hd-columns of Wout).  The out projection is split into two head-pair
halves (j=0,1 / j=2,3) so the tail after the last attention chain is
tiny; the host sums the 4 partials per batch and adds bout.

v4 design (all-bf16 fused pipeline):
  - Everything bf16 on the wire and in SBUF (fp8 was tried and measured:
    softmax averaging does NOT damp relative error, every fp8 link costs
    2-5% L2 vs the 2e-2 max-rel gate).  PSUM accumulates fp32.
  - No phases: q/k/v projection units, attention chains (scores -> exp
    -> att@v), and out-projection halves are emitted interleaved.  Each
    chain carries a *filler* list of projection/out-proj units that are
    emitted two per y-block inside the chain, so the priority-heap Tile
    scheduler always has ready PE work while the scalar engine grinds
    the exp stream (chain ACT 17.7us > chain PE 14us).
  - PSUM (8 banks): 2 rotating score slots [128,2,512] (4 banks), po
    A/B att@v accumulators [65,512] (2 banks), 2 shared [128,512] slots
    for proj/bcast/out-proj (2 banks).
  - softmax denominator comes from a ones-column in the v tile (att @
    v_aug emits Z as row 64); normalization = ones-matmul partition
    broadcast + DVE reciprocal + one in-place multiply.
"""

import sys

sys.path.insert(0, "/opt/trn_rl_repo")

import numpy as np

import concourse.bass as bass
import concourse.mybir as mybir
import concourse.tile as tile
from concourse.bass_utils import run_bass_kernel_spmd

F32 = mybir.dt.float32
BF16 = mybir.dt.bfloat16

HIDDEN = 1024
NUM_HEADS = 16
HEAD_DIM = 64
B = 4
SQ = 1024
SKV = 2048
NCORES = 8
NHL = 8          # heads per core
HL = NHL * HEAD_DIM  # 512, local hd width
SCALE = HEAD_DIM ** -0.5
KCH = HIDDEN // 128  # 8 contraction chunks for the projections
NYC = SKV // 128     # 16 key/value row chunks


def _legalize_waits(nc, limit=1):
    """The walrus build in this container accepts only ~1 sync-wait per
    instruction struct; spill excess waits onto preceding engine NoOps."""
    import bass_rust

    ctr = 0
    for fn in nc.m.functions:
        for blk in fn.blocks:
            out = []
            changed = False
            for inst in blk.instructions:
                si = inst.sync_info
                ws = list(si.on_wait) if si is not None and si.on_wait else []
                if len(ws) > limit:
                    spill, keep = ws[:-limit], ws[-limit:]
                    for w in spill:
                        ctr += 1
                        nop = mybir.InstNoOp(name=f"ant-waitnop-{ctr}", ins=[], outs=[])
                        nop.engine = inst.engine
                        nop.sync_info = bass_rust.SyncInfo(on_wait=[w], on_update=[])
                        out.append(nop)
                    si.on_wait = keep
                    changed = True
                out.append(inst)
            if changed:
                blk.instructions = out
    return ctr


def _emit(nc, tc, debug=False):
    xqT = nc.dram_tensor("xqT", [HIDDEN, SQ], BF16, kind="ExternalInput")
    xkvT = nc.dram_tensor("xkvT", [HIDDEN, SKV], BF16, kind="ExternalInput")
    WqT = nc.dram_tensor("WqT", [HIDDEN, HL], BF16, kind="ExternalInput")
    WkT = nc.dram_tensor("WkT", [HIDDEN, HL], BF16, kind="ExternalInput")
    WvT = nc.dram_tensor("WvT", [HIDDEN, HL], BF16, kind="ExternalInput")
    WoT = nc.dram_tensor("WoT", [HL, HIDDEN], BF16, kind="ExternalInput")
    onesA = nc.dram_tensor("onesA", [1, 128], BF16, kind="ExternalInput")
    onesB = nc.dram_tensor("onesB", [1, 128], BF16, kind="ExternalInput")
    # one partial per head pair j; host sums all of them (+ the other core's)
    out_ds = [
        nc.dram_tensor(f"out{j}", [SQ, HIDDEN], BF16, kind="ExternalOutput")
        for j in range(4)
    ]

    with tc.tile_pool(name="persist", bufs=1) as pp:
        # persistent SBUF (bytes/partition):
        xqT_t = pp.tile([128, KCH, SQ], BF16)      # 16 KB
        xkvT_t = pp.tile([128, KCH, SKV], BF16)    # 32 KB
        WqT_t = pp.tile([128, KCH, HL], BF16)      # 8 KB
        WkT_t = pp.tile([128, KCH, HL], BF16)      # 8 KB
        WvT_t = pp.tile([128, KCH, HL], BF16)      # 8 KB
        WoT_t = pp.tile([128, 4, HIDDEN], BF16)    # 8 KB
        qT = pp.tile([128, 4, SQ], BF16)           # 8 KB
        kT = pp.tile([128, 4, SKV], BF16)          # 16 KB
        va = pp.tile([128, NYC, NHL, 65], BF16)    # 16.3 KB
        xTu = pp.tile([128, 4, SQ], BF16)          # 8 KB
        onesA_t = pp.tile([1, 128], BF16)
        onesB_t = pp.tile([1, 128], BF16)

        # ---- input DMAs: q-side stream on sync, kv-side stream on scalar.
        # Wq/xq are chunk-interleaved so the k-th accumulation step of the
        # first q-proj unit unblocks as soon as its two chunks land.
        nc.sync.dma_start(out=onesA_t, in_=onesA[:, :])
        nc.sync.dma_start(out=onesB_t, in_=onesB[:, :])
        for k in range(KCH):
            nc.sync.dma_start(
                out=WqT_t[:, k, :], in_=WqT[k * 128 : (k + 1) * 128, :]
            )
            nc.sync.dma_start(out=xqT_t[:, k, :], in_=xqT[k * 128 : (k + 1) * 128, :])
        nc.sync.dma_start(out=WoT_t, in_=WoT.rearrange("(j p) n -> p j n", p=128))
        nc.scalar.dma_start(out=WkT_t, in_=WkT.rearrange("(k p) n -> p k n", p=128))
        for k in range(KCH):
            nc.scalar.dma_start(
                out=xkvT_t[:, k, :], in_=xkvT[k * 128 : (k + 1) * 128, :]
            )
        nc.scalar.dma_start(out=WvT_t, in_=WvT.rearrange("(k p) n -> p k n", p=128))

        with (
            tc.tile_pool(name="psS", bufs=1, space="PSUM") as psS,
            tc.tile_pool(name="psPO", bufs=1, space="PSUM") as psPO,
            tc.tile_pool(name="psP", bufs=1, space="PSUM") as psP,
            tc.tile_pool(name="attp", bufs=1) as attp,
            tc.tile_pool(name="zrp", bufs=4) as zrp,
            tc.tile_pool(name="rzp", bufs=2) as rzp,
            tc.tile_pool(name="outst", bufs=3) as outst,
        ):
            nc.vector.memset(va[:, :, :, 64:65], 1.0)

            _pcnt = [0]
            _wide = [True]  # before chain 0's att@v, the PO banks are free

            def p_slot():
                _pcnt[0] += 1
                if _wide[0]:
                    tag = ("P0", "P1", "POA", "POB")[_pcnt[0] % 4]
                    pool = psPO if tag.startswith("PO") else psP
                    return pool.tile([128, 512], F32, tag=tag, name="pslot")
                return psP.tile([128, 512], F32, tag=f"P{_pcnt[0] % 2}", name="pslot")

            def qu(m, s):
                # q projection for head pair m, x window s
                def emit():
                    pt = p_slot()
                    for k in range(KCH):
                        nc.tensor.matmul(
                            pt[:, :],
                            lhsT=WqT_t[:, k, m * 128 : (m + 1) * 128],
                            rhs=xqT_t[:, k, s * 512 : (s + 1) * 512],
                            start=(k == 0),
                            stop=(k == KCH - 1),
                        )
                    nc.vector.tensor_copy(
                        out=qT[:, m, s * 512 : (s + 1) * 512], in_=pt[:, :]
                    )
                return emit

            def ku(m, yg):
                # k projection for head pair m, y window yg
                def emit():
                    pt = p_slot()
                    for k in range(KCH):
                        nc.tensor.matmul(
                            pt[:, :],
                            lhsT=WkT_t[:, k, m * 128 : (m + 1) * 128],
                            rhs=xkvT_t[:, k, yg * 512 : (yg + 1) * 512],
                            start=(k == 0),
                            stop=(k == KCH - 1),
                        )
                    nc.vector.tensor_copy(
                        out=kT[:, m, yg * 512 : (yg + 1) * 512], in_=pt[:, :]
                    )
                return emit

            def vu(half, yc):
                # v projection for heads [4*half, 4*half+4), y chunk yc
                hlo = half * 4

                def emit():
                    pt = p_slot()
                    for k in range(KCH):
                        nc.tensor.matmul(
                            pt[:, 0:256],
                            lhsT=xkvT_t[:, k, yc * 128 : (yc + 1) * 128],
                            rhs=WvT_t[:, k, hlo * 64 : (hlo + 4) * 64],
                            start=(k == 0),
                            stop=(k == KCH - 1),
                        )
                    nc.vector.tensor_copy(
                        out=va[:, yc, hlo : hlo + 4, 0:64],
                        in_=pt[:, 0:256].rearrange("p (h d) -> p h d", h=4),
                    )
                return emit

            def ou(sc, j):
                # single-pair out projection partial for s rows
                # [sc*128, (sc+1)*128); gated only on chain (sc//4, j)
                od = out_ds[j]

                def emit():
                    for oc in range(HIDDEN // 512):
                        pt = p_slot()
                        nc.tensor.matmul(
                            pt[:, :],
                            lhsT=xTu[:, j, sc * 128 : (sc + 1) * 128],
                            rhs=WoT_t[:, j, oc * 512 : (oc + 1) * 512],
                            start=True,
                            stop=True,
                        )
                        ot = outst.tile([128, 512], BF16, tag="out")
                        nc.vector.tensor_copy(out=ot[:, :], in_=pt[:, :])
                        nc.sync.dma_start(
                            out=od[
                                sc * 128 : (sc + 1) * 128, oc * 512 : (oc + 1) * 512
                            ],
                            in_=ot[:, :],
                        )
                return emit

            NYB = NYC // 2  # 8 double-y blocks per chain
            LAG = 3         # attv trails scores/exp by this many blocks
            _scnt = [0]
            _acnt = [0]

            def chain(xc, j, filler=()):
                filler = list(filler)
                hA, hB = 2 * j, 2 * j + 1
                poA = psPO.tile([65, 512], F32, tag="POA", name="poA")
                poB = psPO.tile([65, 512], F32, tag="POB", name="poB")
                attsA, attsB = [], []

                def scores(h, yb):
                    # 4 single-bank score slots + per-yc exps: finer PE<->ACT
                    # pipelining so scores are almost never slot-blocked
                    pq = (h % 2) * 64
                    m = h // 2
                    ats = []
                    for i in range(2):
                        yc = 2 * yb + i
                        _scnt[0] += 1
                        pscr = psS.tile(
                            [128, 512], F32, tag=f"S{_scnt[0] % 4}", name="pscr"
                        )
                        nc.tensor.matmul(
                            pscr[:, :],
                            lhsT=kT[pq : pq + 64, m, yc * 128 : (yc + 1) * 128],
                            rhs=qT[pq : pq + 64, m, xc * 512 : (xc + 1) * 512],
                            start=True,
                            stop=True,
                        )
                        _acnt[0] += 1
                        at = attp.tile(
                            [128, 512], BF16, tag=f"att{_acnt[0] % 12}", name="at"
                        )
                        nc.scalar.activation(
                            out=at[:, :],
                            in_=pscr[:, :],
                            func=mybir.ActivationFunctionType.Exp,
                            scale=SCALE,
                        )
                        ats.append(at)
                    return ats

                def attv(h, po, ats, yb):
                    for i in range(2):
                        yc = 2 * yb + i
                        nc.tensor.matmul(
                            po[:, :],
                            lhsT=va[:, yc, h, :],
                            rhs=ats[i][:, :],
                            start=(yc == 0),
                            stop=(yc == NYC - 1),
                        )

                for yb in range(NYB):
                    # feed the scheduler ready filler work before this block
                    for _ in range(2):
                        if filler:
                            filler.pop(0)()
                    attsA.append(scores(hA, yb))
                    attsB.append(scores(hB, yb))
                    if yb >= LAG:
                        attv(hA, poA, attsA[yb - LAG], yb - LAG)
                        attv(hB, poB, attsB[yb - LAG], yb - LAG)
                for yb in range(NYB - LAG, NYB):
                    attv(hA, poA, attsA[yb], yb)
                    attv(hB, poB, attsB[yb], yb)
                for f in filler:
                    f()

                # Z rows first so the broadcast+reciprocal overlap the xTu
                # copies on the critical tail
                zrs = []
                for hi, po in ((0, poA), (1, poB)):
                    zr = zrp.tile([1, 512], BF16, tag=f"zr{hi}", name="zr")
                    nc.vector.tensor_copy(out=zr[0:1, :], in_=po[64:65, :])
                    zrs.append(zr)
                for hi, po in ((0, poA), (1, poB)):
                    ps_off = hi * 64
                    nc.vector.tensor_copy(
                        out=xTu[ps_off : ps_off + 64, j, xc * 512 : (xc + 1) * 512],
                        in_=po[0:64, :],
                    )
                # broadcast Z of both heads across partitions, reciprocal,
                # then normalize xTu in place.  pb lives in the POA bank: its
                # deps already chain through po, and this keeps the P slots
                # free for projection filler at chain boundaries.
                pb = psPO.tile([128, 512], F32, tag="POA", name="pb")
                nc.tensor.matmul(
                    pb[:, :], lhsT=onesA_t[0:1, :], rhs=zrs[0][0:1, :],
                    start=True, stop=False,
                )
                nc.tensor.matmul(
                    pb[:, :], lhsT=onesB_t[0:1, :], rhs=zrs[1][0:1, :],
                    start=False, stop=True,
                )
                rz = rzp.tile([128, 512], F32, tag="rz", name="rz")
                nc.vector.reciprocal(out=rz[:, :], in_=pb[:, :])
                nc.vector.tensor_mul(
                    xTu[:, j, xc * 512 : (xc + 1) * 512],
                    xTu[:, j, xc * 512 : (xc + 1) * 512],
                    rz[:, :],
                )

            # ---- fused emission order (priority order for the scheduler)
            qu(0, 0)()
            for yg in range(4):
                ku(0, yg)()
            qu(0, 1)()
            _wide[0] = False  # PO banks belong to the attention chains now
            chain(0, 0, filler=[vu(0, yc) for yc in range(NYC)])
            qu(1, 0)()
            for yg in range(4):
                ku(1, yg)()
            qu(1, 1)()
            chain(0, 1, filler=[vu(1, yc) for yc in range(NYC)])
            qu(2, 0)()
            for yg in range(4):
                ku(2, yg)()
            qu(2, 1)()
            chain(0, 2, filler=[qu(3, 0), ku(3, 0), ku(3, 1), ku(3, 2),
                                ku(3, 3), qu(3, 1),
                                ou(0, 0), ou(1, 0), ou(2, 0), ou(3, 0)])
            chain(0, 3, filler=[ou(0, 1), ou(1, 1), ou(2, 1), ou(3, 1)])
            chain(1, 0, filler=[ou(0, 2), ou(1, 2), ou(2, 2), ou(3, 2)])
            chain(1, 1, filler=[ou(0, 3), ou(1, 3), ou(2, 3), ou(3, 3),
                                ou(4, 0), ou(5, 0), ou(6, 0), ou(7, 0)])
            chain(1, 2, filler=[ou(4, 1), ou(5, 1), ou(6, 1), ou(7, 1)])
            chain(1, 3, filler=[ou(4, 2), ou(5, 2), ou(6, 2), ou(7, 2)])
            _wide[0] = True  # PO banks are free again for the tail out-proj
            for sc in range(4, 8):
                ou(sc, 3)()

            if debug:
                qT_d = nc.dram_tensor("qT_d", [128, 4, SQ], BF16, kind="ExternalOutput")
                kT_d = nc.dram_tensor("kT_d", [128, 4, SKV], BF16, kind="ExternalOutput")
                va_d = nc.dram_tensor("va_d", [128, NYC, NHL, 65], BF16, kind="ExternalOutput")
                xTu_d = nc.dram_tensor("xTu_d", [128, 4, SQ], BF16, kind="ExternalOutput")
                nc.sync.dma_start(out=qT_d[:, :, :], in_=qT[:, :, :])
                nc.sync.dma_start(out=kT_d[:, :, :], in_=kT[:, :, :])
                nc.sync.dma_start(out=va_d[:, :, :, :], in_=va[:, :, :, :])
                nc.sync.dma_start(out=xTu_d[:, :, :], in_=xTu[:, :, :])


_NC = None


def _get_nc():
    global _NC
    if _NC is None:
        nc = bass.Bass(trn_type="TRN2")
        with tile.TileContext(nc) as tc:
            _emit(nc, tc)
        _legalize_waits(nc)
        _NC = nc
    return _NC


def _prep_inputs(xq, xkv, Wq, Wkv, Wout):
    import ml_dtypes

    bf = ml_dtypes.bfloat16
    xq = np.asarray(xq, dtype=np.float32)
    xkv = np.asarray(xkv, dtype=np.float32)
    Wq = np.asarray(Wq, dtype=np.float32)
    Wkv = np.asarray(Wkv, dtype=np.float32)
    Wout = np.asarray(Wout, dtype=np.float32)

    onesA = np.zeros((1, 128), bf)
    onesA[0, 0:64] = 1.0
    onesB = np.zeros((1, 128), bf)
    onesB[0, 64:128] = 1.0

    xqT = [np.ascontiguousarray(xq[b].T).astype(bf) for b in range(B)]
    xkvT = [np.ascontiguousarray(xkv[b].T).astype(bf) for b in range(B)]

    per_hg = []
    for hg in range(2):
        hs = slice(hg * HL, (hg + 1) * HL)
        WqTh = np.ascontiguousarray(Wq[hs].T).astype(bf)
        WkTh = np.ascontiguousarray(Wkv[hs].T).astype(bf)
        WvTh = np.ascontiguousarray(
            Wkv[HIDDEN + hg * HL : HIDDEN + (hg + 1) * HL].T
        ).astype(bf)
        WoTh = np.ascontiguousarray(Wout[:, hs].T).astype(bf)
        per_hg.append((WqTh, WkTh, WvTh, WoTh))

    in_maps = []
    for c in range(NCORES):
        b, hg = c // 2, c % 2
        WqTh, WkTh, WvTh, WoTh = per_hg[hg]
        in_maps.append(
            {
                "xqT": xqT[b],
                "xkvT": xkvT[b],
                "WqT": WqTh,
                "WkT": WkTh,
                "WvT": WvTh,
                "WoT": WoTh,
                "onesA": onesA,
                "onesB": onesB,
            }
        )
    return in_maps


def run_sharded(xq, xkv, Wq, Wkv, Wout, bout, trace=False, **kwargs):
    """Build+run the SPMD kernel; returns (full_output, BassKernelResults)."""
    nc = _get_nc()
    in_maps = _prep_inputs(xq, xkv, Wq, Wkv, Wout)
    res = run_bass_kernel_spmd(
        nc, in_maps, core_ids=list(range(NCORES)), trace=trace, **kwargs
    )
    bout = np.asarray(bout, dtype=np.float32)
    out = np.empty((B, SQ, HIDDEN), np.float32)
    for b in range(B):
        acc = np.zeros((SQ, HIDDEN), np.float32)
        for c in (2 * b, 2 * b + 1):
            for j in range(4):
                acc += res.results[c][f"out{j}"].astype(np.float32)
        out[b] = acc
    out += bout[None, None, :]
    return out, res


def kernel(xq, xkv, Wq, Wkv, Wout, bout):
    out, _ = run_sharded(xq, xkv, Wq, Wkv, Wout, bout)
    return out


# revision 36
# speedup vs baseline: 1.2207x; 1.2207x over previous
"""Cross-attention block (nn_CABlock) on 8 TRN2 NeuronCores.

Reference (per batch b):
    q  = xq @ Wq.T            -> [SQ, H]   split heads [SQ, 16, 64]
    kv = xkv @ Wkv.T          -> [SKV, 2H] split [SKV, 2, 16, 64]
    att = softmax(q k^T / sqrt(64))
    x   = att @ v  (merge heads)
    out = x @ Wout.T + bout

Sharding: 8 cores = 4 batches x 2 head-groups (8 heads each).  Each core
computes its batch's projections restricted to its 8 heads, attention for
those heads, and a partial out-projection (contraction over its 512
hd-columns of Wout).  The out projection is split into two head-pair
halves (j=0,1 / j=2,3) so the tail after the last attention chain is
tiny; the host sums the 4 partials per batch and adds bout.

v4 design (all-bf16 fused pipeline):
  - Everything bf16 on the wire and in SBUF (fp8 was tried and measured:
    softmax averaging does NOT damp relative error, every fp8 link costs
    2-5% L2 vs the 2e-2 max-rel gate).  PSUM accumulates fp32.
  - No phases: q/k/v projection units, attention chains (scores -> exp
    -> att@v), and out-projection halves are emitted interleaved.  Each
    chain carries a *filler* list of projection/out-proj units that are
    emitted two per y-block inside the chain, so the priority-heap Tile
    scheduler always has ready PE work while the scalar engine grinds
    the exp stream (chain ACT 17.7us > chain PE 14us).
  - PSUM (8 banks): 2 rotating score slots [128,2,512] (4 banks), po
    A/B att@v accumulators [65,512] (2 banks), 2 shared [128,512] slots
    for proj/bcast/out-proj (2 banks).
  - softmax denominator comes from a ones-column in the v tile (att @
    v_aug emits Z as row 64); normalization = ones-matmul partition
    broadcast + DVE reciprocal + one in-place multiply.
"""

import sys

sys.path.insert(0, "/opt/trn_rl_repo")

import numpy as np

import concourse.bass as bass
import concourse.mybir as mybir
import concourse.tile as tile
from concourse.bass_utils import run_bass_kernel_spmd

F32 = mybir.dt.float32
BF16 = mybir.dt.bfloat16

HIDDEN = 1024
NUM_HEADS = 16
HEAD_DIM = 64
B = 4
SQ = 1024
SKV = 2048
NCORES = 8
NHL = 8          # heads per core
HL = NHL * HEAD_DIM  # 512, local hd width
SCALE = HEAD_DIM ** -0.5
KCH = HIDDEN // 128  # 8 contraction chunks for the projections
NYC = SKV // 128     # 16 key/value row chunks


def _legalize_waits(nc, limit=1):
    """The walrus build in this container accepts only ~1 sync-wait per
    instruction struct; spill excess waits onto preceding engine NoOps."""
    import bass_rust

    ctr = 0
    for fn in nc.m.functions:
        for blk in fn.blocks:
            out = []
            changed = False
            for inst in blk.instructions:
                si = inst.sync_info
                ws = list(si.on_wait) if si is not None and si.on_wait else []
                if len(ws) > limit:
                    spill, keep = ws[:-limit], ws[-limit:]
                    for w in spill:
                        ctr += 1
                        nop = mybir.InstNoOp(name=f"ant-waitnop-{ctr}", ins=[], outs=[])
                        nop.engine = inst.engine
                        nop.sync_info = bass_rust.SyncInfo(on_wait=[w], on_update=[])
                        out.append(nop)
                    si.on_wait = keep
                    changed = True
                out.append(inst)
            if changed:
                blk.instructions = out
    return ctr


def _emit(nc, tc, debug=False):
    xqT = nc.dram_tensor("xqT", [HIDDEN, SQ], BF16, kind="ExternalInput")
    xkvT = nc.dram_tensor("xkvT", [HIDDEN, SKV], BF16, kind="ExternalInput")
    WqT = nc.dram_tensor("WqT", [HIDDEN, HL], BF16, kind="ExternalInput")
    WkT = nc.dram_tensor("WkT", [HIDDEN, HL], BF16, kind="ExternalInput")
    WvT = nc.dram_tensor("WvT", [HIDDEN, HL], BF16, kind="ExternalInput")
    WoT = nc.dram_tensor("WoT", [HL, HIDDEN], BF16, kind="ExternalInput")
    onesA = nc.dram_tensor("onesA", [1, 128], BF16, kind="ExternalInput")
    onesB = nc.dram_tensor("onesB", [1, 128], BF16, kind="ExternalInput")
    # one partial per head pair j; host sums all of them (+ the other core's)
    out_ds = [
        nc.dram_tensor(f"out{j}", [SQ, HIDDEN], BF16, kind="ExternalOutput")
        for j in range(4)
    ]

    with tc.tile_pool(name="persist", bufs=1) as pp:
        # persistent SBUF (bytes/partition):
        xqT_t = pp.tile([128, KCH, SQ], BF16)      # 16 KB
        xkvT_t = pp.tile([128, KCH, SKV], BF16)    # 32 KB
        WqT_t = pp.tile([128, KCH, HL], BF16)      # 8 KB
        WkT_t = pp.tile([128, KCH, HL], BF16)      # 8 KB
        WvT_t = pp.tile([128, KCH, HL], BF16)      # 8 KB
        WoT_t = pp.tile([128, 4, HIDDEN], BF16)    # 8 KB
        qT = pp.tile([128, 4, SQ], BF16)           # 8 KB
        kT = pp.tile([128, 4, SKV], BF16)          # 16 KB
        va = pp.tile([128, NYC, NHL, 65], BF16)    # 16.3 KB
        xTu = pp.tile([128, 4, SQ], BF16)          # 8 KB
        onesA_t = pp.tile([1, 128], BF16)
        onesB_t = pp.tile([1, 128], BF16)

        # ---- input DMAs: q-side stream on sync, kv-side stream on scalar.
        # Wq/xq are chunk-interleaved so the k-th accumulation step of the
        # first q-proj unit unblocks as soon as its two chunks land.
        nc.sync.dma_start(out=onesA_t, in_=onesA[:, :])
        nc.sync.dma_start(out=onesB_t, in_=onesB[:, :])
        for k in range(KCH):
            nc.sync.dma_start(
                out=WqT_t[:, k, :], in_=WqT[k * 128 : (k + 1) * 128, :]
            )
            nc.sync.dma_start(out=xqT_t[:, k, :], in_=xqT[k * 128 : (k + 1) * 128, :])
        nc.sync.dma_start(out=WoT_t, in_=WoT.rearrange("(j p) n -> p j n", p=128))
        nc.scalar.dma_start(out=WkT_t, in_=WkT.rearrange("(k p) n -> p k n", p=128))
        for k in range(KCH):
            nc.scalar.dma_start(
                out=xkvT_t[:, k, :], in_=xkvT[k * 128 : (k + 1) * 128, :]
            )
        nc.scalar.dma_start(out=WvT_t, in_=WvT.rearrange("(k p) n -> p k n", p=128))

        with (
            tc.tile_pool(name="psS", bufs=1, space="PSUM") as psS,
            tc.tile_pool(name="psPO", bufs=1, space="PSUM") as psPO,
            tc.tile_pool(name="psP", bufs=1, space="PSUM") as psP,
            tc.tile_pool(name="attp", bufs=1) as attp,
            tc.tile_pool(name="zrp", bufs=4) as zrp,
            tc.tile_pool(name="rzp", bufs=2) as rzp,
            tc.tile_pool(name="outst", bufs=3) as outst,
        ):
            nc.vector.memset(va[:, :, :, 64:65], 1.0)

            _pcnt = [0]
            _wide = [True]  # before chain 0's att@v, the PO banks are free

            def p_slot():
                _pcnt[0] += 1
                if _wide[0]:
                    tag = ("P0", "P1", "POA", "POB")[_pcnt[0] % 4]
                    pool = psPO if tag.startswith("PO") else psP
                    return pool.tile([128, 512], F32, tag=tag, name="pslot")
                return psP.tile([128, 512], F32, tag=f"P{_pcnt[0] % 2}", name="pslot")

            def qu(m, s):
                # q projection for head pair m, x window s
                def emit():
                    pt = p_slot()
                    for k in range(KCH):
                        nc.tensor.matmul(
                            pt[:, :],
                            lhsT=WqT_t[:, k, m * 128 : (m + 1) * 128],
                            rhs=xqT_t[:, k, s * 512 : (s + 1) * 512],
                            start=(k == 0),
                            stop=(k == KCH - 1),
                        )
                    nc.vector.tensor_copy(
                        out=qT[:, m, s * 512 : (s + 1) * 512], in_=pt[:, :]
                    )
                return emit

            def ku(m, yg):
                # k projection for head pair m, y window yg
                def emit():
                    pt = p_slot()
                    for k in range(KCH):
                        nc.tensor.matmul(
                            pt[:, :],
                            lhsT=WkT_t[:, k, m * 128 : (m + 1) * 128],
                            rhs=xkvT_t[:, k, yg * 512 : (yg + 1) * 512],
                            start=(k == 0),
                            stop=(k == KCH - 1),
                        )
                    nc.vector.tensor_copy(
                        out=kT[:, m, yg * 512 : (yg + 1) * 512], in_=pt[:, :]
                    )
                return emit

            def vu(half, yc):
                # v projection for heads [4*half, 4*half+4), y chunk yc
                hlo = half * 4

                def emit():
                    pt = p_slot()
                    for k in range(KCH):
                        nc.tensor.matmul(
                            pt[:, 0:256],
                            lhsT=xkvT_t[:, k, yc * 128 : (yc + 1) * 128],
                            rhs=WvT_t[:, k, hlo * 64 : (hlo + 4) * 64],
                            start=(k == 0),
                            stop=(k == KCH - 1),
                        )
                    nc.vector.tensor_copy(
                        out=va[:, yc, hlo : hlo + 4, 0:64],
                        in_=pt[:, 0:256].rearrange("p (h d) -> p h d", h=4),
                    )
                return emit

            def ou(sc, j):
                # single-pair out projection partial for s rows
                # [sc*128, (sc+1)*128); gated only on chain (sc//4, j)
                od = out_ds[j]

                def emit():
                    for oc in range(HIDDEN // 512):
                        pt = p_slot()
                        nc.tensor.matmul(
                            pt[:, :],
                            lhsT=xTu[:, j, sc * 128 : (sc + 1) * 128],
                            rhs=WoT_t[:, j, oc * 512 : (oc + 1) * 512],
                            start=True,
                            stop=True,
                        )
                        ot = outst.tile([128, 512], BF16, tag="out")
                        nc.vector.tensor_copy(out=ot[:, :], in_=pt[:, :])
                        nc.sync.dma_start(
                            out=od[
                                sc * 128 : (sc + 1) * 128, oc * 512 : (oc + 1) * 512
                            ],
                            in_=ot[:, :],
                        )
                return emit

            NYB = NYC // 2  # 8 double-y blocks per chain
            LAG = 3         # attv trails scores/exp by this many blocks
            _scnt = [0]
            _acnt = [0]

            def chain(xc, j, filler=()):
                filler = list(filler)
                hA, hB = 2 * j, 2 * j + 1
                poA = psPO.tile([65, 512], F32, tag="POA", name="poA")
                poB = psPO.tile([65, 512], F32, tag="POB", name="poB")
                attsA, attsB = [], []

                def scores(h, yb):
                    pq = (h % 2) * 64
                    m = h // 2
                    _scnt[0] += 1
                    pscr = psS.tile(
                        [128, 2, 512], F32, tag=f"S{_scnt[0] % 2}", name="pscr"
                    )
                    for i in range(2):
                        yc = 2 * yb + i
                        nc.tensor.matmul(
                            pscr[:, i, :],
                            lhsT=kT[pq : pq + 64, m, yc * 128 : (yc + 1) * 128],
                            rhs=qT[pq : pq + 64, m, xc * 512 : (xc + 1) * 512],
                            start=True,
                            stop=True,
                        )
                    _acnt[0] += 1
                    at = attp.tile(
                        [128, 2, 512], BF16, tag=f"att{_acnt[0] % 10}", name="at"
                    )
                    nc.scalar.activation(
                        out=at[:, :, :].rearrange("p a b -> p (a b)"),
                        in_=pscr[:, :, :].rearrange("p a b -> p (a b)"),
                        func=mybir.ActivationFunctionType.Exp,
                        scale=SCALE,
                    )
                    return at

                def attv(h, po, at, yb):
                    for i in range(2):
                        yc = 2 * yb + i
                        nc.tensor.matmul(
                            po[:, :],
                            lhsT=va[:, yc, h, :],
                            rhs=at[:, i, :],
                            start=(yc == 0),
                            stop=(yc == NYC - 1),
                        )

                for yb in range(NYB):
                    # feed the scheduler ready filler work before this block
                    for _ in range(2):
                        if filler:
                            filler.pop(0)()
                    attsA.append(scores(hA, yb))
                    attsB.append(scores(hB, yb))
                    if yb >= LAG:
                        attv(hA, poA, attsA[yb - LAG], yb - LAG)
                        attv(hB, poB, attsB[yb - LAG], yb - LAG)
                for yb in range(NYB - LAG, NYB):
                    attv(hA, poA, attsA[yb], yb)
                    attv(hB, poB, attsB[yb], yb)
                for f in filler:
                    f()

                # Z rows first so the broadcast+reciprocal overlap the xTu
                # copies on the critical tail
                zrs = []
                for hi, po in ((0, poA), (1, poB)):
                    zr = zrp.tile([1, 512], BF16, tag=f"zr{hi}", name="zr")
                    nc.vector.tensor_copy(out=zr[0:1, :], in_=po[64:65, :])
                    zrs.append(zr)
                for hi, po in ((0, poA), (1, poB)):
                    ps_off = hi * 64
                    nc.vector.tensor_copy(
                        out=xTu[ps_off : ps_off + 64, j, xc * 512 : (xc + 1) * 512],
                        in_=po[0:64, :],
                    )
                # broadcast Z of both heads across partitions, reciprocal,
                # then normalize xTu in place.  pb lives in the POA bank: its
                # deps already chain through po, and this keeps the P slots
                # free for projection filler at chain boundaries.
                pb = psPO.tile([128, 512], F32, tag="POA", name="pb")
                nc.tensor.matmul(
                    pb[:, :], lhsT=onesA_t[0:1, :], rhs=zrs[0][0:1, :],
                    start=True, stop=False,
                )
                nc.tensor.matmul(
                    pb[:, :], lhsT=onesB_t[0:1, :], rhs=zrs[1][0:1, :],
                    start=False, stop=True,
                )
                rz = rzp.tile([128, 512], F32, tag="rz", name="rz")
                nc.vector.reciprocal(out=rz[:, :], in_=pb[:, :])
                nc.vector.tensor_mul(
                    xTu[:, j, xc * 512 : (xc + 1) * 512],
                    xTu[:, j, xc * 512 : (xc + 1) * 512],
                    rz[:, :],
                )

            # ---- fused emission order (priority order for the scheduler)
            qu(0, 0)()
            for yg in range(4):
                ku(0, yg)()
            qu(0, 1)()
            _wide[0] = False  # PO banks belong to the attention chains now
            chain(0, 0, filler=[vu(0, yc) for yc in range(NYC)])
            qu(1, 0)()
            for yg in range(4):
                ku(1, yg)()
            qu(1, 1)()
            chain(0, 1, filler=[vu(1, yc) for yc in range(NYC)])
            qu(2, 0)()
            for yg in range(4):
                ku(2, yg)()
            qu(2, 1)()
            chain(0, 2, filler=[qu(3, 0), ku(3, 0), ku(3, 1), ku(3, 2),
                                ku(3, 3), qu(3, 1),
                                ou(0, 0), ou(1, 0), ou(2, 0), ou(3, 0)])
            chain(0, 3, filler=[ou(0, 1), ou(1, 1), ou(2, 1), ou(3, 1)])
            chain(1, 0, filler=[ou(0, 2), ou(1, 2), ou(2, 2), ou(3, 2)])
            chain(1, 1, filler=[ou(0, 3), ou(1, 3), ou(2, 3), ou(3, 3),
                                ou(4, 0), ou(5, 0), ou(6, 0), ou(7, 0)])
            chain(1, 2, filler=[ou(4, 1), ou(5, 1), ou(6, 1), ou(7, 1)])
            chain(1, 3, filler=[ou(4, 2), ou(5, 2), ou(6, 2), ou(7, 2)])
            _wide[0] = True  # PO banks are free again for the tail out-proj
            for sc in range(4, 8):
                ou(sc, 3)()

            if debug:
                qT_d = nc.dram_tensor("qT_d", [128, 4, SQ], BF16, kind="ExternalOutput")
                kT_d = nc.dram_tensor("kT_d", [128, 4, SKV], BF16, kind="ExternalOutput")
                va_d = nc.dram_tensor("va_d", [128, NYC, NHL, 65], BF16, kind="ExternalOutput")
                xTu_d = nc.dram_tensor("xTu_d", [128, 4, SQ], BF16, kind="ExternalOutput")
                nc.sync.dma_start(out=qT_d[:, :, :], in_=qT[:, :, :])
                nc.sync.dma_start(out=kT_d[:, :, :], in_=kT[:, :, :])
                nc.sync.dma_start(out=va_d[:, :, :, :], in_=va[:, :, :, :])
                nc.sync.dma_start(out=xTu_d[:, :, :], in_=xTu[:, :, :])


_NC = None


def _get_nc():
    global _NC
    if _NC is None:
        nc = bass.Bass(trn_type="TRN2")
        with tile.TileContext(nc) as tc:
            _emit(nc, tc)
        _legalize_waits(nc)
        _NC = nc
    return _NC


def _prep_inputs(xq, xkv, Wq, Wkv, Wout):
    import ml_dtypes

    bf = ml_dtypes.bfloat16
    xq = np.asarray(xq, dtype=np.float32)
    xkv = np.asarray(xkv, dtype=np.float32)
    Wq = np.asarray(Wq, dtype=np.float32)
    Wkv = np.asarray(Wkv, dtype=np.float32)
    Wout = np.asarray(Wout, dtype=np.float32)

    onesA = np.zeros((1, 128), bf)
    onesA[0, 0:64] = 1.0
    onesB = np.zeros((1, 128), bf)
    onesB[0, 64:128] = 1.0

    xqT = [np.ascontiguousarray(xq[b].T).astype(bf) for b in range(B)]
    xkvT = [np.ascontiguousarray(xkv[b].T).astype(bf) for b in range(B)]

    per_hg = []
    for hg in range(2):
        hs = slice(hg * HL, (hg + 1) * HL)
        WqTh = np.ascontiguousarray(Wq[hs].T).astype(bf)
        WkTh = np.ascontiguousarray(Wkv[hs].T).astype(bf)
        WvTh = np.ascontiguousarray(
            Wkv[HIDDEN + hg * HL : HIDDEN + (hg + 1) * HL].T
        ).astype(bf)
        WoTh = np.ascontiguousarray(Wout[:, hs].T).astype(bf)
        per_hg.append((WqTh, WkTh, WvTh, WoTh))

    in_maps = []
    for c in range(NCORES):
        b, hg = c // 2, c % 2
        WqTh, WkTh, WvTh, WoTh = per_hg[hg]
        in_maps.append(
            {
                "xqT": xqT[b],
                "xkvT": xkvT[b],
                "WqT": WqTh,
                "WkT": WkTh,
                "WvT": WvTh,
                "WoT": WoTh,
                "onesA": onesA,
                "onesB": onesB,
            }
        )
    return in_maps


def run_sharded(xq, xkv, Wq, Wkv, Wout, bout, trace=False, **kwargs):
    """Build+run the SPMD kernel; returns (full_output, BassKernelResults)."""
    nc = _get_nc()
    in_maps = _prep_inputs(xq, xkv, Wq, Wkv, Wout)
    res = run_bass_kernel_spmd(
        nc, in_maps, core_ids=list(range(NCORES)), trace=trace, **kwargs
    )
    bout = np.asarray(bout, dtype=np.float32)
    out = np.empty((B, SQ, HIDDEN), np.float32)
    for b in range(B):
        acc = np.zeros((SQ, HIDDEN), np.float32)
        for c in (2 * b, 2 * b + 1):
            for j in range(4):
                acc += res.results[c][f"out{j}"].astype(np.float32)
        out[b] = acc
    out += bout[None, None, :]
    return out, res


def kernel(xq, xkv, Wq, Wkv, Wout, bout):
    out, _ = run_sharded(xq, xkv, Wq, Wkv, Wout, bout)
    return out
